# revision 19
# baseline (speedup 1.0000x reference)
"""Cross-attention (B=2, N=2048, M=4096, H=16, dh=64) on 8 TRN2 NeuronCores.

Collective-free SPMD design. Host→device transfer happens outside the hot
loop (device_put of the sharded inputs), so inputs are REPLICATED per batch
instead of wire-lean sharded: core c (b=c//4, g=c%4) receives the full
pre-transposed activations of batch b plus only its head-group g weights:

  xT   = x[b].T        [1024, 2048] f16   (pre-transposed on host)
  ctxT = context[b].T  [768, 4096]  f16
  wq   [128, 8*256]  f16  wq[p, fi*256+j]   = Wq[fi*128+p, 256g+j]
  wkv  [128, 6*512]  f16  wkv[p, fi*512+j]  = [Wk|Wv][fi*128+p, cols 256g..]
  wo   [128, 2*1024] f16  wo[p, pair*1024+j]= Wo[256g+128*pair+p, j]
  lm   [128, 32]     f32  additive log-mask, column per 128-wide m-tile

No on-device collectives or transposes remain: each core computes the full
[2048, 1024] partial output of its head group (fp32 PSUM accumulation
everywhere, fp16 operands) and the host sums the four head-group partials
per batch during unshard (+bo).

Per-core pipeline (PE at full fp16 rate):
  qT[d, n]   = Wq_g^T @ x^T    (PSUM acc over 8 feature tiles)
  kT[d, m], v[m, d] from ctx^T per 512-wide m-chunk; the attention stream
  for (head 0, n-chunk 0) and the exps of (head 1, n-chunk 0) are chased
  inside the m-chunk loop so the ACT exp stream starts immediately.
  S^T[m-tile, n] = k_h^T.T @ q_h^T             (PSUM f32)
  P^T = exp(S^T * scale + logmask[m])          (ACT, PSUM->SBUF f16)
  acc[65, n] += [v_h | 1].T @ P^T              (PSUM acc over m-tiles;
                                                row 64 = softmax denom)
  o^T_h = acc[0:64] * (1/acc[64])              (DVE + gpsimd bcast)
  out[n-tile] = sum_pairs o^T.T @ Wo_g         (head-group partial, DMA out)
"""

from contextlib import ExitStack
from functools import lru_cache

import numpy as np

import concourse.bass as bass
import concourse.mybir as mybir
import concourse.tile as tile
from concourse import bacc
from concourse.bass_utils import run_bass_kernel_spmd

F16 = mybir.dt.float16
F32 = mybir.dt.float32
AF = mybir.ActivationFunctionType

N_CORES = 8
B, N, M = 2, 2048, 4096
DQ, DC = 1024, 768          # query dim, context dim
H, DH = 16, 64              # total heads, head dim
HPC = 4                     # heads per core
GD = HPC * DH               # 256 inner dims per core
SCALE = DH ** -0.5
NEG = -30000.0              # additive mask value for masked-out positions

FQ = DQ // 128              # 8 feature tiles of x
FC = DC // 128              # 6 feature tiles of ctx
NT = N // 128               # 16 query tiles
MT = M // 128               # 32 context tiles
VW = DH + 1                 # 65: v columns + ones column
NC2 = 1024                  # n-chunk of the attention pipeline


def build_kernel(tc: tile.TileContext, ins: dict, outs: dict):
    nc = tc.nc
    xT_d, ctxT_d = ins["xT"], ins["ctxT"]
    wq_d, wkv_d, wo_d, lm_d = ins["wq"], ins["wkv"], ins["wo"], ins["lm"]
    out_d = outs["out"]

    es = ExitStack()
    with es:
        const = es.enter_context(tc.tile_pool(name="const", bufs=1))
        wpool = es.enter_context(tc.tile_pool(name="weights", bufs=1))
        persist = es.enter_context(tc.tile_pool(name="persist", bufs=1))

        # DMA issue order tracks consumer order: lm+wq gate the first work,
        # wkv is needed ~15us in (issued mid x-stream), wo only at the first
        # out-projection (~170us in, issued after phase B).
        lm_sb = const.tile([128, MT], F32)
        nc.sync.dma_start(out=lm_sb, in_=lm_d)
        wq_sb = wpool.tile([128, FQ * GD], F16)
        nc.sync.dma_start(out=wq_sb, in_=wq_d)
        wkv_sb = wpool.tile([128, FC * 2 * GD], F16)
        wo_sb = wpool.tile([128, 2 * DQ], F16)

        # preload the Exp activation table concurrently with the input DMAs
        # so the first real exp doesn't eat the table-load latency
        warm = const.tile([1, 2], F32)
        nc.vector.memset(warm, 0.0)
        warm_o = const.tile([1, 2], F32)
        nc.scalar.activation(out=warm_o, in_=warm, func=AF.Exp)

        kT_sb = persist.tile([128, 2, M], F16)     # [d within pair, pair, m]
        v_sb = persist.tile([128, MT, HPC, VW], F16)
        qT_sb = persist.tile([128, 2, N], F16)
        oT_sb = persist.tile([128, 2, N], F16)
        # exp(scores) of the chased second head (h=1, n-chunk 0), produced
        # inside the ctx loop to fill ACT idle gaps; attnV rides in unit (2,0)
        pT10_sb = persist.tile([128, MT, NC2], F16)

        # ones columns of v (softmax denominator accumulators)
        ones32 = const.tile([128, 1], F32)
        nc.vector.memset(ones32, 1.0)
        nc.vector.tensor_copy(
            out=v_sb[:, :, :, DH:DH + 1],
            in_=ones32.unsqueeze(1).unsqueeze(1).to_broadcast([128, MT, HPC, 1]))

        # attention pipeline pools are opened after phase A (PSUM budget)
        es2 = es.enter_context(ExitStack())
        st_psum = acc_psum = p_pool = div_pool = None

        def scores_exp(h, ncK, mt, pT):
            """scores -> exp for one (head, n-chunk, m-tile) into pT."""
            pair, ro = divmod(h, 2)
            ro *= DH
            st = st_psum.tile([128, NC2], F32, tag="st")
            for hf in range(NC2 // 512):
                nc.tensor.matmul(
                    st[:, hf * 512:(hf + 1) * 512],
                    kT_sb[ro:ro + DH, pair, mt * 128:(mt + 1) * 128],
                    qT_sb[ro:ro + DH, pair,
                          ncK * NC2 + hf * 512:ncK * NC2 + (hf + 1) * 512],
                    start=True, stop=True)
            nc.scalar.activation(
                out=pT, in_=st, func=AF.Exp,
                bias=lm_sb[:, mt:mt + 1], scale=SCALE)

        def attn_v(h, mt, pT, acc, start=None, stop=None):
            """acc += [v_h | 1].T @ pT (PSUM accumulate across m-tiles).
            start/stop default to m-order; pass explicitly when the
            accumulation is executed out of m-order."""
            if start is None:
                start = (mt == 0)
            if stop is None:
                stop = (mt == MT - 1)
            for hf in range(NC2 // 512):
                nc.tensor.matmul(
                    acc[:, hf * 512:(hf + 1) * 512],
                    v_sb[:, mt, h, :],
                    pT[:, hf * 512:(hf + 1) * 512],
                    start=start, stop=stop)

        def attn_mt(h, ncK, mt, acc):
            """scores -> exp -> attnV for one (head, n-chunk, m-tile)."""
            pT = p_pool.tile([128, NC2], F16, tag="pT")
            scores_exp(h, ncK, mt, pT)
            attn_v(h, mt, pT, acc)

        def divide(h, ncK, acc, spill=True):
            """o^T_h = acc[0:64] / acc[64] into oT_sb.

            With spill=True the PSUM accumulator is first copied to SBUF so
            its bank frees after one DVE copy instead of after the whole
            recip->broadcast->mul chain (the next unit's first attnV waits
            on that release)."""
            pair, ro = divmod(h, 2)
            ro *= DH
            if spill:
                sp = div_pool.tile([VW, NC2], F32, tag="sp")
                nc.vector.tensor_copy(out=sp, in_=acc)
                acc = sp
            rec = div_pool.tile([1, NC2], F32, tag="rec")
            nc.vector.reciprocal(out=rec, in_=acc[DH:DH + 1, :])
            bc = div_pool.tile([DH, NC2], F32, tag="bc")
            nc.gpsimd.partition_broadcast(bc, rec)
            nc.vector.tensor_mul(
                out=oT_sb[ro:ro + DH, pair, ncK * NC2:(ncK + 1) * NC2],
                in0=acc[0:DH, :], in1=bc)

        # ---------------- xT -> qT ----------------
        # x streams in [128, 512] column chunks so the first matmul series
        # starts ~3us in instead of waiting for full-row DMAs to drain.
        with (
            tc.tile_pool(name="xld", bufs=2 * FQ) as xld,
            tc.tile_pool(name="mm_ps", bufs=4, space="PSUM") as mm_psum,
        ):
            for nq in range(N // 512):
                x_tiles = []
                for fi in range(FQ):
                    t = xld.tile([128, 512], F16, tag="x")
                    nc.sync.dma_start(
                        out=t,
                        in_=xT_d[fi * 128:(fi + 1) * 128,
                                 nq * 512:(nq + 1) * 512])
                    x_tiles.append(t)
                if nq == 1:  # wkv lands well before the first kT matmul
                    nc.sync.dma_start(out=wkv_sb, in_=wkv_d)
                for pair in range(2):
                    ps = mm_psum.tile([128, 512], F32, tag="mm")
                    for fi in range(FQ):
                        nc.tensor.matmul(
                            ps,
                            wq_sb[:, fi * GD + pair * 128:
                                  fi * GD + (pair + 1) * 128],
                            x_tiles[fi],
                            start=(fi == 0), stop=(fi == FQ - 1))
                    nc.vector.tensor_copy(
                        out=qT_sb[:, pair, nq * 512:(nq + 1) * 512], in_=ps)

        # attention pipeline pools used by the chase + pure units
        st_psum = es2.enter_context(
            tc.tile_pool(name="st_ps", bufs=2, space="PSUM"))
        p_pool = es2.enter_context(tc.tile_pool(name="pT", bufs=6))
        div_pool = es2.enter_context(tc.tile_pool(name="div", bufs=1))

        # ---------------- ctxT -> kT, v (+ chase of h0/h1, chunk 0) ----------
        with (
            tc.tile_pool(name="cld", bufs=12) as cld,
            tc.tile_pool(name="a_ps", bufs=2, space="PSUM") as a_psum,
            tc.tile_pool(name="acc0_ps", bufs=1, space="PSUM") as acc0_psum,
        ):
            acc00 = acc0_psum.tile([VW, NC2], F32, tag="acc0")
            for mc in range(M // 512):
                ctx_tiles = []
                for fi in range(FC):
                    t = cld.tile([128, 512], F16, tag="c")
                    nc.sync.dma_start(
                        out=t,
                        in_=ctxT_d[fi * 128:(fi + 1) * 128,
                                   mc * 512:(mc + 1) * 512])
                    ctx_tiles.append(t)
                for pair in range(2):  # k^T d-tiles
                    ps = a_psum.tile([128, 512], F32, tag="a")
                    for fi in range(FC):
                        nc.tensor.matmul(
                            ps,
                            wkv_sb[:, fi * 2 * GD + pair * 128:
                                   fi * 2 * GD + (pair + 1) * 128],
                            ctx_tiles[fi],
                            start=(fi == 0), stop=(fi == FC - 1))
                    nc.vector.tensor_copy(
                        out=kT_sb[:, pair, mc * 512:(mc + 1) * 512], in_=ps)
                for s in range(4):  # v m-subtiles
                    mt = mc * 4 + s
                    ps = a_psum.tile([128, GD], F32, tag="a")
                    for fi in range(FC):
                        nc.tensor.matmul(
                            ps,
                            ctx_tiles[fi][:, s * 128:(s + 1) * 128],
                            wkv_sb[:, fi * 2 * GD + GD:fi * 2 * GD + 2 * GD],
                            start=(fi == 0), stop=(fi == FC - 1))
                    # one strided copy lands all four heads' v columns
                    # (the ones columns at [..., DH] are pre-set and skipped)
                    nc.vector.tensor_copy(
                        out=v_sb[:, mt, :, 0:DH],
                        in_=ps.rearrange("p (h w) -> p h w", h=HPC))
                # chase: full (h0, nc0) attention + (h1, nc0) exps over this
                # chunk's m-tiles keep the ACT exp stream hot; h1's attnV
                # accumulation is deferred to unit (2,0)'s loop.
                for s in range(4):
                    mt = mc * 4 + s
                    attn_mt(0, 0, mt, acc00)
                    scores_exp(1, 0, mt, pT10_sb[:, mt, :])
            divide(0, 0, acc00)

        # -------- remaining attention units + interleaved out-projection -----
        # PSUM banks in this phase: st 2x2 + acc 1x2 + op 1x2 = 8. The
        # out-projection gets its own pool (op) so it never steals a score
        # slot and stalls the sc->exp double-buffer; the deferred h1
        # accumulator borrows the op slot (it is released, via divide(1,0),
        # before the first out_proj allocation needs it).
        acc_psum = es2.enter_context(
            tc.tile_pool(name="acc_ps", bufs=1, space="PSUM"))
        op_psum = es2.enter_context(
            tc.tile_pool(name="op_ps", bufs=1, space="PSUM"))
        fin_pool = es2.enter_context(tc.tile_pool(name="fin_sb", bufs=4))
        nc.sync.dma_start(out=wo_sb, in_=wo_d)

        def out_proj(nt, psum=None, tag="op", act_copy=False):
            ps = (psum or op_psum).tile([128, NC2], F32, tag=tag)
            for hf in range(DQ // 512):
                for pair in range(2):
                    nc.tensor.matmul(
                        ps[:, hf * 512:(hf + 1) * 512],
                        oT_sb[:, pair, nt * 128:(nt + 1) * 128],
                        wo_sb[:, pair * DQ + hf * 512:
                              pair * DQ + (hf + 1) * 512],
                        start=(pair == 0), stop=(pair == 1))
            fs = fin_pool.tile([128, NC2], F16, tag="fs")
            if act_copy:  # tail: ACT and DVE copy halves in parallel
                nc.scalar.copy(out=fs[:, 0:512], in_=ps[:, 0:512])
                nc.vector.tensor_copy(out=fs[:, 512:NC2], in_=ps[:, 512:NC2])
            else:
                nc.vector.tensor_copy(out=fs, in_=ps)
            nc.sync.dma_start(
                out=out_d[nt * 128:(nt + 1) * 128, :], in_=fs)

        def attn_unit(h, ncK, extra=None):
            """One (head, n-chunk) unit; `extra(mt)` interleaves PE-side work
            (deferred attnV / out-proj) between m-tiles without breaking the
            ACT exp stream."""
            acc = acc_psum.tile([VW, NC2], F32, tag="acc")
            for mt in range(MT):
                attn_mt(h, ncK, mt, acc)
                if extra is not None:
                    extra(mt)
            divide(h, ncK, acc)

        # head 1 / chunk 0: exps were produced in the ctx loop; its deferred
        # attnV accumulation is split across units (2,0) (odd m-tiles) and
        # (3,0) (even m-tiles) so both units stay at the ACT exp pace instead
        # of one being PE-bound. Execution order: mt=1 is the first
        # accumulation, mt=30 the last — flags passed explicitly.
        acc10 = op_psum.tile([VW, NC2], F32, tag="op")
        attn_unit(2, 0, extra=lambda mt: attn_v(
            1, mt, pT10_sb[:, mt, :], acc10, start=(mt == 1), stop=False)
            if mt % 2 else None)
        attn_unit(3, 0, extra=lambda mt: attn_v(
            1, mt, pT10_sb[:, mt, :], acc10, start=False, stop=(mt == 30))
            if mt % 2 == 0 else None)
        divide(1, 0, acc10)
        # unit (0,1): the first-half out-projection rides inside the loop
        acc01 = acc_psum.tile([VW, NC2], F32, tag="acc")
        for mt in range(MT):
            attn_mt(0, 1, mt, acc01)
            if mt % 4 == 3:
                out_proj(mt // 4)
        divide(0, 1, acc01)
        attn_unit(1, 1)
        attn_unit(2, 1)
        acc31 = acc_psum.tile([VW, NC2], F32, tag="acc")
        for mt in range(MT):
            attn_mt(3, 1, mt, acc31)
        divide(3, 1, acc31, spill=False)
        # tail projections double-buffer across the op and (now idle) st
        # PSUM pools, with the PSUM->SBUF copies on the idle ACT engine
        for i, nt in enumerate(range(NT // 2, NT)):
            if i % 2:
                out_proj(nt, psum=st_psum, tag="st", act_copy=True)
            else:
                out_proj(nt, act_copy=True)
        es2.close()


@lru_cache(maxsize=4)
def build_program(iters: int = 1):
    nc = bacc.Bacc("TRN2", target_bir_lowering=False, debug=False,
                   num_devices=N_CORES)
    ins = {
        "xT": nc.dram_tensor("xT", [DQ, N], F16, kind="ExternalInput").ap(),
        "ctxT": nc.dram_tensor("ctxT", [DC, M], F16,
                               kind="ExternalInput").ap(),
        "wq": nc.dram_tensor("wq", [128, FQ * GD], F16,
                             kind="ExternalInput").ap(),
        "wkv": nc.dram_tensor("wkv", [128, FC * 2 * GD], F16,
                              kind="ExternalInput").ap(),
        "wo": nc.dram_tensor("wo", [128, 2 * DQ], F16,
                             kind="ExternalInput").ap(),
        "lm": nc.dram_tensor("lm", [128, MT], F32, kind="ExternalInput").ap(),
    }
    outs = {
        "out": nc.dram_tensor("out", [N, DQ], F16, kind="ExternalOutput").ap(),
    }
    with tile.TileContext(nc) as tc:
        for _ in range(iters):
            build_kernel(tc, ins, outs)
    nc.compile()
    return nc


def make_in_maps(x, context, context_mask, Wq, Wk, Wv, Wo):
    Wq, Wk, Wv, Wo = (np.asarray(a) for a in (Wq, Wk, Wv, Wo))
    xT = [np.ascontiguousarray(np.asarray(x[b]).T, dtype=np.float16)
          for b in range(B)]
    ctxT = [np.ascontiguousarray(np.asarray(context[b]).T, dtype=np.float16)
            for b in range(B)]
    lm = [np.ascontiguousarray(
        np.where(context_mask[b], 0.0, NEG).astype(np.float32)
        .reshape(MT, 128).T) for b in range(B)]
    in_maps = []
    for c in range(N_CORES):
        b, g = divmod(c, HPC)
        gs = slice(g * GD, (g + 1) * GD)
        wq = (Wq[:, gs].reshape(FQ, 128, GD).transpose(1, 0, 2)
              .reshape(128, FQ * GD))
        wkv = np.concatenate([Wk[:, gs], Wv[:, gs]], axis=1)  # [768, 512]
        wkv = (wkv.reshape(FC, 128, 2 * GD).transpose(1, 0, 2)
               .reshape(128, FC * 2 * GD))
        wo = (Wo[gs].reshape(2, 128, DQ).transpose(1, 0, 2)
              .reshape(128, 2 * DQ))
        in_maps.append({
            "xT": xT[b], "ctxT": ctxT[b],
            "wq": np.ascontiguousarray(wq, dtype=np.float16),
            "wkv": np.ascontiguousarray(wkv, dtype=np.float16),
            "wo": np.ascontiguousarray(wo, dtype=np.float16),
            "lm": lm[b],
        })
    return in_maps


def assemble_output(results, bo):
    # Each core returns its head-group partial of the full [N, DQ] output;
    # the four partials per batch are summed here during unshard.
    out = np.zeros((B, N, DQ), np.float32)
    for c in range(N_CORES):
        b = c // HPC
        out[b] += results[c]["out"].astype(np.float32)
    out += np.asarray(bo, np.float32)
    return out


@lru_cache(maxsize=4)
def _runner(iters: int = 1):
    """Persistent jitted executor (same path run_bass_kernel_spmd takes under
    axon, but cached so repeated kernel() calls skip re-trace/re-lowering,
    and the constant zero output buffers stay device-resident)."""
    import jax
    from jax.sharding import Mesh, NamedSharding, PartitionSpec
    from jax.experimental.shard_map import shard_map
    from concourse import bass2jax

    nc = build_program(iters)
    bass2jax.install_neuronx_cc_hook()
    partition_name = (nc.partition_id_tensor.name
                      if nc.partition_id_tensor else None)
    in_names, out_names, out_avals, zero_outs = [], [], [], []
    for alloc in nc.m.functions[0].allocations:
        if not isinstance(alloc, mybir.MemoryLocationSet):
            continue
        name = alloc.memorylocations[0].name
        if alloc.kind == "ExternalInput":
            if name != partition_name:
                in_names.append(name)
        elif alloc.kind == "ExternalOutput":
            out_names.append(name)
            shape = tuple(alloc.tensor_shape)
            dtype = mybir.dt.np(alloc.dtype)
            out_avals.append(jax.core.ShapedArray(shape, dtype))
            zero_outs.append(np.zeros(shape, dtype))
    n_params = len(in_names)
    all_names = list(in_names) + out_names
    if partition_name is not None:
        all_names.append(partition_name)

    def _body(*args):
        operands = list(args)
        if partition_name is not None:
            operands.append(bass2jax.partition_id_tensor())
        return tuple(bass2jax._bass_exec_p.bind(
            *operands, out_avals=tuple(out_avals), in_names=tuple(all_names),
            out_names=tuple(out_names), lowering_input_output_aliases=(),
            sim_require_finite=True, sim_require_nnan=True, nc=nc))

    mesh = Mesh(np.asarray(jax.devices()[:N_CORES]), ("core",))
    specs = (PartitionSpec("core"),) * (n_params + len(out_avals))
    sharded = jax.jit(
        shard_map(_body, mesh=mesh, in_specs=specs,
                  out_specs=(PartitionSpec("core"),) * len(out_avals),
                  check_rep=False),
        keep_unused=True)
    sh = NamedSharding(mesh, PartitionSpec("core"))
    dev_zeros = [
        jax.device_put(np.zeros((N_CORES * z.shape[0], *z.shape[1:]), z.dtype),
                       sh)
        for z in zero_outs]

    def run(in_maps):
        concat_in = [
            jax.device_put(np.concatenate(
                [np.asarray(in_maps[c][name]) for c in range(N_CORES)],
                axis=0), sh)
            for name in in_names]
        out_arrs = sharded(*concat_in, *dev_zeros)
        return [
            {name: np.asarray(out_arrs[i]).reshape(
                N_CORES, *out_avals[i].shape)[c]
             for i, name in enumerate(out_names)}
            for c in range(N_CORES)]

    return run


def kernel(**inputs):
    x = np.asarray(inputs["x"], np.float32)
    context = np.asarray(inputs["context"], np.float32)
    mask = np.asarray(inputs["context_mask"])
    in_maps = make_in_maps(x, context, mask,
                           inputs["Wq"], inputs["Wk"], inputs["Wv"],
                           inputs["Wo"])
    try:
        results = _runner(1)(in_maps)
    except Exception:
        res = run_bass_kernel_spmd(build_program(1), in_maps,
                                   core_ids=list(range(N_CORES)))
        results = res.results
    return assemble_output(results, inputs["bo"])


if __name__ == "__main__":
    rng = np.random.default_rng(0)
    ins = {
        "x": rng.normal(size=(B, N, DQ)).astype(np.float32),
        "context": rng.normal(size=(B, M, DC)).astype(np.float32),
        "context_mask": np.ones((B, M), bool),
        "Wq": (rng.normal(size=(DQ, H * DH)) * 0.02).astype(np.float32),
        "Wk": (rng.normal(size=(DC, H * DH)) * 0.02).astype(np.float32),
        "Wv": (rng.normal(size=(DC, H * DH)) * 0.02).astype(np.float32),
        "Wo": (rng.normal(size=(H * DH, DQ)) * 0.02).astype(np.float32),
        "bo": np.zeros((DQ,), np.float32),
    }
    out = kernel(**ins)
    print("out", out.shape, out.dtype, float(np.abs(out).mean()))

    try:  # local dev check only; reference.py is absent in the grading dir
        import reference as ref
    except ImportError:
        ref = None
    if ref is not None:
        exp = np.asarray(
            ref.reference(**{k: np.asarray(v) for k, v in ins.items()}))
        rel = float(np.abs(out - exp).max() / np.abs(exp).max())
        print("local rel err vs reference:", rel)
